# revision 17
# baseline (speedup 1.0000x reference)
"""Multi-head attention (B=2, S=2048, D=1024, H=16) on 8 TRN2 NeuronCores.

Sharding: tensor-parallel over heads. Each core owns 2 heads (128 feature
channels): Wq/Wk/Wv sliced column-wise (rows of the [out,in] weight), Wo
sliced row-wise. x/y replicated. Each core emits a partial [4096, 1024]
output (its heads pushed through its Wo slice); host sums the 8 partials.

Device-side layout trick: everything is computed transposed-by-design so no
on-device transposes of the big activations are needed:
  - host feeds x^T, y^T  [1024, 4096]
  - Q^T/K^T = (W x^T)        [128 chan, 4096 tok]   (chan on partitions)
  - scores  S^T = K^T.T-slices @ Q^T  -> [kpos, q]  (softmax axis = partitions)
  - E = exp(S^T) unnormalized; denominator comes for free as a 65th "ones"
    column in the V stationary operand of the attn@V matmul
  - O_un^T [d, q] = [V|1].T @ E
  - normalize with a gpsimd partition-broadcast of 1/denom
  - out = O_norm^T.T-slices @ Wo^T   (q back on partitions)
Matmuls run as float32r (TF32-ish, 1 cycle/row at N>=256). Scores pack the
two heads into the PE array with row tiling (contraction=64 each).
"""

import os
import numpy as np
from contextlib import ExitStack

# Problem constants (hardcoded per contract; kernel.py must be self-contained)
B, S, D = 2, 2048, 1024
T = B * S            # 4096 flattened tokens
N_CORES = 8
DC = D // N_CORES    # 128 channels per core
HD = 64              # head dim
NH = DC // HD        # 2 heads per core
SCALE = 1.0 / np.sqrt(HD)  # folded into Wq/bq on host
KT_TILES = S // 128  # 16 key tiles per batch
QC = 512             # query chunk (matmul moving N)
NQC = S // QC        # 4 query chunks per batch

_CACHE = {}


def _get_nc():
    if "nc" in _CACHE:
        return _CACHE["nc"]
    import concourse.bass as bass
    import concourse.mybir as mybir
    import concourse.tile as tile
    from concourse import bacc

    f32 = mybir.dt.float32
    f32r = mybir.dt.float32r
    PSUM = bass.MemorySpace.PSUM

    nc = bacc.Bacc(
        "TRN2",
        target_bir_lowering=False,
        debug=False,
        enable_asserts=False,
        num_devices=N_CORES,
    )

    xT_d = nc.dram_tensor("xT", [D, T], f32r, kind="ExternalInput").ap()
    yT_d = nc.dram_tensor("yT", [D, T], f32r, kind="ExternalInput").ap()
    wqT_d = nc.dram_tensor("wqT", [D, DC], f32r, kind="ExternalInput").ap()
    wkT_d = nc.dram_tensor("wkT", [D, DC], f32r, kind="ExternalInput").ap()
    wvT_d = nc.dram_tensor("wvT", [D, DC], f32r, kind="ExternalInput").ap()
    woT_d = nc.dram_tensor("woT", [DC, D], f32r, kind="ExternalInput").ap()
    bq_d = nc.dram_tensor("bq", [DC, 1], f32, kind="ExternalInput").ap()
    bk_d = nc.dram_tensor("bk", [DC, 1], f32, kind="ExternalInput").ap()
    bv_d = nc.dram_tensor("bv", [DC, 1], f32, kind="ExternalInput").ap()
    ident_d = nc.dram_tensor("ident", [128, 128], f32, kind="ExternalInput").ap()
    vones_d = nc.dram_tensor("vones", [128, 64], f32r, kind="ExternalInput").ap()
    out_d = nc.dram_tensor("out", [T, D], f32, kind="ExternalOutput").ap()

    VW = 2 * (HD + 1)  # 130: per key-tile V layout [V_h0 | 1 | V_h1 | 1]

    with tile.TileContext(nc) as tc, ExitStack() as top:
        persist = top.enter_context(tc.tile_pool(name="persist", bufs=1))

        # ---- persistent SBUF tensors ----
        wq_sb = persist.tile([128, D], f32r, tag="wq")    # [din-part, 8*128 chan]
        wk_sb = persist.tile([128, D], f32r, tag="wk")
        wv_sb = persist.tile([128, D], f32r, tag="wv")
        wo_sb = persist.tile([DC, D], f32r, tag="wo")     # [d-part, 1024 out]
        bq_sb = persist.tile([DC, 1], f32, tag="bq")
        bk_sb = persist.tile([DC, 1], f32, tag="bk")
        bv_sb = persist.tile([DC, 1], f32, tag="bv")
        id_sb = persist.tile([128, 128], f32, tag="ident")
        ones64_sb = persist.tile([1, HD], f32r, tag="ones64")
        qT_sb = persist.tile([DC, T], f32r, tag="qT")     # [chan, tok]
        kT_sb = persist.tile([DC, T], f32r, tag="kT")
        vT_sb = persist.tile([DC, T], f32, tag="vT")
        v_all = persist.tile([128, (T // 128) * VW], f32r, tag="vall")

        for i in range(D // 128):
            nc.sync.dma_start(wq_sb[:, i * 128:(i + 1) * 128], wqT_d[i * 128:(i + 1) * 128, :])
            nc.sync.dma_start(wk_sb[:, i * 128:(i + 1) * 128], wkT_d[i * 128:(i + 1) * 128, :])
            nc.sync.dma_start(wv_sb[:, i * 128:(i + 1) * 128], wvT_d[i * 128:(i + 1) * 128, :])
        nc.sync.dma_start(wo_sb[:], woT_d[:])
        nc.sync.dma_start(bq_sb[:], bq_d[:])
        nc.sync.dma_start(bk_sb[:], bk_d[:])
        nc.sync.dma_start(bv_sb[:], bv_d[:])
        nc.sync.dma_start(id_sb[:], ident_d[:])

        # ones columns of v_all (denominator rider rows)
        v3 = v_all[:].rearrange("p (t c) -> p t c", c=VW)
        vones_3d = vones_d[:, 0:T // 128].rearrange("p (t o) -> p t o", o=1)
        nc.sync.dma_start(v3[:, :, HD:HD + 1], vones_3d)
        nc.sync.dma_start(v3[:, :, 2 * HD + 1:2 * HD + 2], vones_3d)
        nc.sync.dma_start(ones64_sb[:], vones_d[0:1, 0:HD])

        # ---- phase 1: projections ----
        with ExitStack() as ph1:
            io_pool = ph1.enter_context(tc.tile_pool(name="io", bufs=6))
            pr_ps = ph1.enter_context(tc.tile_pool(name="prps", bufs=4, space=PSUM))
            vt_ps = ph1.enter_context(tc.tile_pool(name="vtps", bufs=2, space=PSUM))

            for tcn in range(T // QC):  # 8 token chunks of 512
                t0 = tcn * QC
                # Q^T chunk
                q_ps = pr_ps.tile([DC, QC], f32, tag="prps")
                for di in range(D // 128):
                    xt = io_pool.tile([128, QC], f32r, tag="io")
                    nc.sync.dma_start(xt[:], xT_d[di * 128:(di + 1) * 128, t0:t0 + QC])
                    nc.tensor.matmul(
                        q_ps[:],
                        wq_sb[:, di * 128:(di + 1) * 128],
                        xt[:],
                        start=(di == 0), stop=(di == D // 128 - 1),
                    )
                nc.vector.tensor_scalar_add(qT_sb[:, t0:t0 + QC], q_ps[:], bq_sb[:])
                # K^T and V^T chunks share the y tiles
                k_ps = pr_ps.tile([DC, QC], f32, tag="prps")
                v_ps = pr_ps.tile([DC, QC], f32, tag="prps")
                for di in range(D // 128):
                    yt = io_pool.tile([128, QC], f32r, tag="io")
                    nc.sync.dma_start(yt[:], yT_d[di * 128:(di + 1) * 128, t0:t0 + QC])
                    nc.tensor.matmul(
                        k_ps[:],
                        wk_sb[:, di * 128:(di + 1) * 128],
                        yt[:],
                        start=(di == 0), stop=(di == D // 128 - 1),
                    )
                    nc.tensor.matmul(
                        v_ps[:],
                        wv_sb[:, di * 128:(di + 1) * 128],
                        yt[:],
                        start=(di == 0), stop=(di == D // 128 - 1),
                    )
                nc.vector.tensor_scalar_add(kT_sb[:, t0:t0 + QC], k_ps[:], bk_sb[:])
                nc.vector.tensor_scalar_add(vT_sb[:, t0:t0 + QC], v_ps[:], bv_sb[:])

            # V^T -> V (PE transpose per 128x128 tile) into v_all
            for kt in range(T // 128):  # 32 key tiles over all tokens
                tp = vt_ps.tile([128, 128], f32, tag="vtps")
                nc.tensor.transpose(tp[:], vT_sb[:, kt * 128:(kt + 1) * 128], id_sb[:])
                for h in range(NH):
                    c0 = kt * VW + h * (HD + 1)
                    nc.vector.tensor_copy(v_all[:, c0:c0 + HD], tp[:, h * HD:(h + 1) * HD])

        # ---- phase 2: attention + Wo ----
        with ExitStack() as ph2:
            s_pool = ph2.enter_context(tc.tile_pool(name="sps", bufs=4, space=PSUM))
            o_pool = ph2.enter_context(tc.tile_pool(name="ops", bufs=2, space=PSUM))
            w_pool = ph2.enter_context(tc.tile_pool(name="wps", bufs=2, space=PSUM))
            e_pool = ph2.enter_context(tc.tile_pool(name="e", bufs=4))
            ou_pool = ph2.enter_context(tc.tile_pool(name="ou", bufs=3))
            rec_pool = ph2.enter_context(tc.tile_pool(name="rec", bufs=2))
            on_pool = ph2.enter_context(tc.tile_pool(name="on", bufs=2))
            st_pool = ph2.enter_context(tc.tile_pool(name="st", bufs=3))

            Exp = mybir.ActivationFunctionType.Exp
            Copy = mybir.ActivationFunctionType.Copy

            for b in range(B):
                for qc in range(NQC):
                    q0 = b * S + qc * QC
                    o_ps = [o_pool.tile([HD + 1, QC], f32, tag="ops", name=f"ops{b}_{qc}_{h}")
                            for h in range(NH)]
                    for kt in range(KT_TILES):
                        k0 = b * S + kt * 128
                        s_ps = []
                        for h in range(NH):
                            sp = s_pool.tile([128, QC], f32, tag="sps", name=f"sps{h}")
                            nc.tensor.matmul(
                                sp[:],
                                kT_sb[h * HD:(h + 1) * HD, k0:k0 + 128],
                                qT_sb[h * HD:(h + 1) * HD, q0:q0 + QC],
                                start=True, stop=True,
                                tile_position=(h * HD, 0),
                            )
                            s_ps.append(sp)
                        for h in range(NH):
                            e_sb = e_pool.tile([128, QC], f32r, tag="e")
                            nc.scalar.activation(e_sb[:], s_ps[h][:], Exp)
                            c0 = (b * KT_TILES + kt) * VW + h * (HD + 1)
                            nc.tensor.matmul(
                                o_ps[h][:],
                                v_all[:, c0:c0 + HD + 1],
                                e_sb[:],
                                start=(kt == 0), stop=(kt == KT_TILES - 1),
                            )
                    # normalize: O_norm^T[d, q] = O_un^T[d, q] / denom[q]
                    on_sb = on_pool.tile([DC, QC], f32r, tag="on")
                    for h in range(NH):
                        ou_sb = ou_pool.tile([HD + 1, QC], f32, tag="ou")
                        nc.scalar.activation(ou_sb[:], o_ps[h][:], Copy)
                        rec_sb = rec_pool.tile([1, QC], f32r, tag="rec",
                                               name=f"rec{b}_{qc}_{h}")
                        with nc.allow_low_precision(reason="softmax denom recip to f32r"):
                            nc.vector.reciprocal(rec_sb[:], ou_sb[HD:HD + 1, :])
                        rb_ps = w_pool.tile([128, QC], f32, tag="wps",
                                            name=f"rb{b}_{qc}_{h}")
                        nc.tensor.matmul(rb_ps[0:HD, :], ones64_sb[:], rec_sb[:],
                                         start=True, stop=True)
                        nc.vector.tensor_mul(
                            on_sb[h * HD:(h + 1) * HD, :], ou_sb[0:HD, :],
                            rb_ps[0:HD, :]
                        )
                    # Wo: out[q, n] = sum_d O_norm^T[d, q] * woT[d, n]
                    for qs in range(QC // 128):
                        st = st_pool.tile([128, D], f32, tag="st")
                        for nn in range(D // QC):
                            wp = w_pool.tile([128, QC], f32, tag="wps")
                            nc.tensor.matmul(
                                wp[:],
                                on_sb[:, qs * 128:(qs + 1) * 128],
                                wo_sb[:, nn * QC:(nn + 1) * QC],
                                start=True, stop=True,
                            )
                            nc.vector.tensor_copy(st[:, nn * QC:(nn + 1) * QC], wp[:])
                        r0 = q0 + qs * 128
                        nc.sync.dma_start(out_d[r0:r0 + 128, :], st[:])

    nc.compile()
    _CACHE["nc"] = nc
    return nc


def _prep_in_maps(x, y, Wq, bq, Wk, bk, Wv, bv, Wo):
    xT = np.ascontiguousarray(x.reshape(T, D).T, dtype=np.float32)
    yT = np.ascontiguousarray(y.reshape(T, D).T, dtype=np.float32)
    ident = np.eye(128, dtype=np.float32)
    in_maps = []
    for c in range(N_CORES):
        sl = slice(c * DC, (c + 1) * DC)
        in_maps.append({
            "xT": xT,
            "yT": yT,
            "wqT": np.ascontiguousarray(Wq[sl].T * SCALE, dtype=np.float32),
            "wkT": np.ascontiguousarray(Wk[sl].T, dtype=np.float32),
            "wvT": np.ascontiguousarray(Wv[sl].T, dtype=np.float32),
            "woT": np.ascontiguousarray(Wo[:, sl].T, dtype=np.float32),
            "bq": np.ascontiguousarray((bq[sl] * SCALE).reshape(DC, 1), dtype=np.float32),
            "bk": np.ascontiguousarray(bk[sl].reshape(DC, 1), dtype=np.float32),
            "bv": np.ascontiguousarray(bv[sl].reshape(DC, 1), dtype=np.float32),
            "ident": ident,
            "vones": np.ones((128, 64), dtype=np.float32),
        })
    return in_maps


def _run(in_maps, trace=False):
    if os.environ.get("JAX_PLATFORMS", "").strip() == "cpu":
        os.environ.pop("JAX_PLATFORMS")
    nc = _get_nc()
    from concourse.bass_utils import run_bass_kernel_spmd
    return run_bass_kernel_spmd(nc, in_maps, core_ids=list(range(N_CORES)), trace=trace)


def _numpy_fallback(x, y, mask, Wq, bq, Wk, bk, Wv, bv, Wo, bo):
    Bs, Sq, Dm = x.shape
    H = 16
    q = (x @ Wq.T + bq).reshape(Bs, Sq, H, HD)
    k = (y @ Wk.T + bk).reshape(Bs, -1, H, HD)
    v = (y @ Wv.T + bv).reshape(Bs, -1, H, HD)
    score = np.einsum("bqhd,bkhd->bhqk", q, k) / np.sqrt(HD)
    score = score + (1.0 - mask[:, None, :, :]) * -1e9
    score -= score.max(axis=-1, keepdims=True)
    e = np.exp(score)
    attn = e / e.sum(axis=-1, keepdims=True)
    out = np.einsum("bhqk,bkhd->bqhd", attn, v).reshape(Bs, Sq, Dm)
    return (out @ Wo.T + bo).astype(np.float32)


def kernel(x, y, mask, Wq, bq, Wk, bk, Wv, bv, Wo, bo):
    x = np.asarray(x, dtype=np.float32)
    y = np.asarray(y, dtype=np.float32)
    mask = np.asarray(mask, dtype=np.float32)
    Wq = np.asarray(Wq, dtype=np.float32)
    bq = np.asarray(bq, dtype=np.float32)
    Wk = np.asarray(Wk, dtype=np.float32)
    bk = np.asarray(bk, dtype=np.float32)
    Wv = np.asarray(Wv, dtype=np.float32)
    bv = np.asarray(bv, dtype=np.float32)
    Wo = np.asarray(Wo, dtype=np.float32)
    bo = np.asarray(bo, dtype=np.float32)

    if not np.all(mask == 1.0):
        return _numpy_fallback(x, y, mask, Wq, bq, Wk, bk, Wv, bv, Wo, bo)

    in_maps = _prep_in_maps(x, y, Wq, bq, Wk, bk, Wv, bv, Wo)
    res = _run(in_maps, trace=False)
    total = res.results[0]["out"].astype(np.float32).copy()
    for c in range(1, N_CORES):
        total += res.results[c]["out"]
    total += bo
    return total.reshape(B, S, D).astype(np.float32)


# revision 20
# speedup vs baseline: 189.5292x; 189.5292x over previous
"""Multi-head attention (B=2, S=2048, D=1024, H=16) on 8 TRN2 NeuronCores.

Sharding: tensor-parallel over heads. Each core owns 2 heads (128 feature
channels): Wq/Wk/Wv sliced column-wise (rows of the [out,in] weight), Wo
sliced row-wise. x/y replicated. Each core emits a partial [4096, 1024]
output (its heads pushed through its Wo slice); host sums the 8 partials.

Device-side layout trick: everything is computed transposed-by-design so no
on-device transposes of the big activations are needed:
  - host feeds x^T, y^T  [1024, 4096]
  - Q^T/K^T = (W x^T)        [128 chan, 4096 tok]   (chan on partitions)
  - scores  S^T = K^T.T-slices @ Q^T  -> [kpos, q]  (softmax axis = partitions)
  - E = exp(S^T) unnormalized; denominator comes for free as a 65th "ones"
    column in the V stationary operand of the attn@V matmul
  - O_un^T [d, q] = [V|1].T @ E
  - normalize with a gpsimd partition-broadcast of 1/denom
  - out = O_norm^T.T-slices @ Wo^T   (q back on partitions)
Matmuls run as float32r (TF32-ish, 1 cycle/row at N>=256). Scores pack the
two heads into the PE array with row tiling (contraction=64 each).
"""

import os
import numpy as np
from contextlib import ExitStack

# Problem constants (hardcoded per contract; kernel.py must be self-contained)
B, S, D = 2, 2048, 1024
T = B * S            # 4096 flattened tokens
N_CORES = 8
DC = D // N_CORES    # 128 channels per core
HD = 64              # head dim
NH = DC // HD        # 2 heads per core
SCALE = 1.0 / np.sqrt(HD)  # folded into Wq/bq on host
KT_TILES = S // 128  # 16 key tiles per batch
QC = 512             # query chunk (matmul moving N)
NQC = S // QC        # 4 query chunks per batch

_CACHE = {}


def _get_nc(reps=1):
    key = f"nc{reps}"
    if key in _CACHE:
        return _CACHE[key]
    import concourse.bass as bass
    import concourse.mybir as mybir
    import concourse.tile as tile
    from concourse import bacc

    f32 = mybir.dt.float32
    f32r = mybir.dt.float32r
    PSUM = bass.MemorySpace.PSUM

    nc = bacc.Bacc(
        "TRN2",
        target_bir_lowering=False,
        debug=False,
        enable_asserts=False,
        num_devices=N_CORES,
    )

    xT_d = nc.dram_tensor("xT", [D, T], f32r, kind="ExternalInput").ap()
    yT_d = nc.dram_tensor("yT", [D, T], f32r, kind="ExternalInput").ap()
    wqT_d = nc.dram_tensor("wqT", [D, DC], f32r, kind="ExternalInput").ap()
    wkT_d = nc.dram_tensor("wkT", [D, DC], f32r, kind="ExternalInput").ap()
    wvT_d = nc.dram_tensor("wvT", [D, DC], f32r, kind="ExternalInput").ap()
    woT_d = nc.dram_tensor("woT", [DC, D], f32r, kind="ExternalInput").ap()
    bq_d = nc.dram_tensor("bq", [DC, 1], f32, kind="ExternalInput").ap()
    bk_d = nc.dram_tensor("bk", [DC, 1], f32, kind="ExternalInput").ap()
    bv_d = nc.dram_tensor("bv", [DC, 1], f32, kind="ExternalInput").ap()
    ident_d = nc.dram_tensor("ident", [128, 128], f32, kind="ExternalInput").ap()
    vones_d = nc.dram_tensor("vones", [128, 64], f32r, kind="ExternalInput").ap()
    out_d = nc.dram_tensor("out", [T, D], f32, kind="ExternalOutput").ap()

    VW = 2 * (HD + 1)  # 130: per key-tile V layout [V_h0 | 1 | V_h1 | 1]

    with tile.TileContext(nc) as tc, ExitStack() as top:
        persist = top.enter_context(tc.tile_pool(name="persist", bufs=1))

        # ---- persistent SBUF tensors ----
        wq_sb = persist.tile([128, D], f32r, tag="wq")    # [din-part, 8*128 chan]
        wk_sb = persist.tile([128, D], f32r, tag="wk")
        wv_sb = persist.tile([128, D], f32r, tag="wv")
        wo_sb = persist.tile([DC, D], f32r, tag="wo")     # [d-part, 1024 out]
        bq_sb = persist.tile([DC, 1], f32, tag="bq")
        bk_sb = persist.tile([DC, 1], f32, tag="bk")
        bv_sb = persist.tile([DC, 1], f32, tag="bv")
        id_sb = persist.tile([128, 128], f32, tag="ident")
        ones64_sb = persist.tile([1, HD], f32r, tag="ones64")
        qT_sb = persist.tile([DC, T], f32r, tag="qT")     # [chan, tok]
        kT_sb = persist.tile([DC, T], f32r, tag="kT")
        vT_sb = persist.tile([DC, T], f32, tag="vT")
        v_all = persist.tile([128, (T // 128) * VW], f32r, tag="vall")

        for i in range(D // 128):
            nc.sync.dma_start(wq_sb[:, i * 128:(i + 1) * 128], wqT_d[i * 128:(i + 1) * 128, :])
            nc.sync.dma_start(wk_sb[:, i * 128:(i + 1) * 128], wkT_d[i * 128:(i + 1) * 128, :])
            nc.sync.dma_start(wv_sb[:, i * 128:(i + 1) * 128], wvT_d[i * 128:(i + 1) * 128, :])
        nc.sync.dma_start(wo_sb[:], woT_d[:])
        nc.sync.dma_start(bq_sb[:], bq_d[:])
        nc.sync.dma_start(bk_sb[:], bk_d[:])
        nc.sync.dma_start(bv_sb[:], bv_d[:])
        nc.sync.dma_start(id_sb[:], ident_d[:])

        # ones columns of v_all (denominator rider rows)
        v3 = v_all[:].rearrange("p (t c) -> p t c", c=VW)
        vones_3d = vones_d[:, 0:T // 128].rearrange("p (t o) -> p t o", o=1)
        nc.sync.dma_start(v3[:, :, HD:HD + 1], vones_3d)
        nc.sync.dma_start(v3[:, :, 2 * HD + 1:2 * HD + 2], vones_3d)
        nc.sync.dma_start(ones64_sb[:], vones_d[0:1, 0:HD])

        # ---- phase 1: projections ----
        for _rep in range(reps):
            _build_body(nc, tc, mybir, bass, locals())

    nc.compile()
    _CACHE[key] = nc
    return nc


def _build_body(nc, tc, mybir, bass, env):
    f32 = mybir.dt.float32
    f32r = mybir.dt.float32r
    PSUM = bass.MemorySpace.PSUM
    xT_d, yT_d = env["xT_d"], env["yT_d"]
    out_d = env["out_d"]
    wq_sb, wk_sb, wv_sb, wo_sb = env["wq_sb"], env["wk_sb"], env["wv_sb"], env["wo_sb"]
    bq_sb, bk_sb, bv_sb = env["bq_sb"], env["bk_sb"], env["bv_sb"]
    id_sb, ones64_sb = env["id_sb"], env["ones64_sb"]
    qT_sb, kT_sb, vT_sb, v_all = env["qT_sb"], env["kT_sb"], env["vT_sb"], env["v_all"]
    VW = 2 * (HD + 1)

    if True:
        with ExitStack() as ph1:
            io_pool = ph1.enter_context(tc.tile_pool(name="io", bufs=6))
            pr_ps = ph1.enter_context(tc.tile_pool(name="prps", bufs=4, space=PSUM))
            vt_ps = ph1.enter_context(tc.tile_pool(name="vtps", bufs=2, space=PSUM))

            for tcn in range(T // QC):  # 8 token chunks of 512
                t0 = tcn * QC
                # Q^T chunk
                q_ps = pr_ps.tile([DC, QC], f32, tag="prps")
                for di in range(D // 128):
                    xt = io_pool.tile([128, QC], f32r, tag="io")
                    nc.sync.dma_start(xt[:], xT_d[di * 128:(di + 1) * 128, t0:t0 + QC])
                    nc.tensor.matmul(
                        q_ps[:],
                        wq_sb[:, di * 128:(di + 1) * 128],
                        xt[:],
                        start=(di == 0), stop=(di == D // 128 - 1),
                    )
                nc.vector.tensor_scalar_add(qT_sb[:, t0:t0 + QC], q_ps[:], bq_sb[:])
                # K^T and V^T chunks share the y tiles
                k_ps = pr_ps.tile([DC, QC], f32, tag="prps")
                v_ps = pr_ps.tile([DC, QC], f32, tag="prps")
                for di in range(D // 128):
                    yt = io_pool.tile([128, QC], f32r, tag="io")
                    nc.sync.dma_start(yt[:], yT_d[di * 128:(di + 1) * 128, t0:t0 + QC])
                    nc.tensor.matmul(
                        k_ps[:],
                        wk_sb[:, di * 128:(di + 1) * 128],
                        yt[:],
                        start=(di == 0), stop=(di == D // 128 - 1),
                    )
                    nc.tensor.matmul(
                        v_ps[:],
                        wv_sb[:, di * 128:(di + 1) * 128],
                        yt[:],
                        start=(di == 0), stop=(di == D // 128 - 1),
                    )
                nc.vector.tensor_scalar_add(kT_sb[:, t0:t0 + QC], k_ps[:], bk_sb[:])
                nc.vector.tensor_scalar_add(vT_sb[:, t0:t0 + QC], v_ps[:], bv_sb[:])

            # V^T -> V (PE transpose per 128x128 tile) into v_all
            for kt in range(T // 128):  # 32 key tiles over all tokens
                tp = vt_ps.tile([128, 128], f32, tag="vtps")
                nc.tensor.transpose(tp[:], vT_sb[:, kt * 128:(kt + 1) * 128], id_sb[:])
                for h in range(NH):
                    c0 = kt * VW + h * (HD + 1)
                    nc.vector.tensor_copy(v_all[:, c0:c0 + HD], tp[:, h * HD:(h + 1) * HD])

        # ---- phase 2: attention + Wo ----
        with ExitStack() as ph2:
            s_pool = ph2.enter_context(tc.tile_pool(name="sps", bufs=4, space=PSUM))
            o_pool = ph2.enter_context(tc.tile_pool(name="ops", bufs=2, space=PSUM))
            w_pool = ph2.enter_context(tc.tile_pool(name="wps", bufs=2, space=PSUM))
            e_pool = ph2.enter_context(tc.tile_pool(name="e", bufs=4))
            ou_pool = ph2.enter_context(tc.tile_pool(name="ou", bufs=3))
            rec_pool = ph2.enter_context(tc.tile_pool(name="rec", bufs=2))
            on_pool = ph2.enter_context(tc.tile_pool(name="on", bufs=2))
            st_pool = ph2.enter_context(tc.tile_pool(name="st", bufs=3))

            Exp = mybir.ActivationFunctionType.Exp
            Copy = mybir.ActivationFunctionType.Copy

            for b in range(B):
                for qc in range(NQC):
                    q0 = b * S + qc * QC
                    o_ps = [o_pool.tile([HD + 1, QC], f32, tag="ops", name=f"ops{b}_{qc}_{h}")
                            for h in range(NH)]
                    for kt in range(KT_TILES):
                        k0 = b * S + kt * 128
                        s_ps = []
                        for h in range(NH):
                            sp = s_pool.tile([128, QC], f32, tag="sps", name=f"sps{h}")
                            nc.tensor.matmul(
                                sp[:],
                                kT_sb[h * HD:(h + 1) * HD, k0:k0 + 128],
                                qT_sb[h * HD:(h + 1) * HD, q0:q0 + QC],
                                start=True, stop=True,
                                tile_position=(h * HD, 0),
                            )
                            s_ps.append(sp)
                        for h in range(NH):
                            e_sb = e_pool.tile([128, QC], f32r, tag="e")
                            nc.scalar.activation(e_sb[:], s_ps[h][:], Exp)
                            c0 = (b * KT_TILES + kt) * VW + h * (HD + 1)
                            nc.tensor.matmul(
                                o_ps[h][:],
                                v_all[:, c0:c0 + HD + 1],
                                e_sb[:],
                                start=(kt == 0), stop=(kt == KT_TILES - 1),
                            )
                    # normalize: O_norm^T[d, q] = O_un^T[d, q] / denom[q]
                    on_sb = on_pool.tile([DC, QC], f32r, tag="on")
                    for h in range(NH):
                        ou_sb = ou_pool.tile([HD + 1, QC], f32, tag="ou")
                        nc.scalar.activation(ou_sb[:], o_ps[h][:], Copy)
                        rec_sb = rec_pool.tile([1, QC], f32r, tag="rec",
                                               name=f"rec{b}_{qc}_{h}")
                        with nc.allow_low_precision(reason="softmax denom recip to f32r"):
                            nc.vector.reciprocal(rec_sb[:], ou_sb[HD:HD + 1, :])
                        rb_ps = w_pool.tile([128, QC], f32, tag="wps",
                                            name=f"rb{b}_{qc}_{h}")
                        nc.tensor.matmul(rb_ps[0:HD, :], ones64_sb[:], rec_sb[:],
                                         start=True, stop=True)
                        nc.vector.tensor_mul(
                            on_sb[h * HD:(h + 1) * HD, :], ou_sb[0:HD, :],
                            rb_ps[0:HD, :]
                        )
                    # Wo: out[q, n] = sum_d O_norm^T[d, q] * woT[d, n]
                    for qs in range(QC // 128):
                        st = st_pool.tile([128, D], f32, tag="st")
                        for nn in range(D // QC):
                            wp = w_pool.tile([128, QC], f32, tag="wps")
                            nc.tensor.matmul(
                                wp[:],
                                on_sb[:, qs * 128:(qs + 1) * 128],
                                wo_sb[:, nn * QC:(nn + 1) * QC],
                                start=True, stop=True,
                            )
                            nc.vector.tensor_copy(st[:, nn * QC:(nn + 1) * QC], wp[:])
                        r0 = q0 + qs * 128
                        nc.sync.dma_start(out_d[r0:r0 + 128, :], st[:])

def _prep_in_maps(x, y, Wq, bq, Wk, bk, Wv, bv, Wo):
    xT = np.ascontiguousarray(x.reshape(T, D).T, dtype=np.float32)
    yT = np.ascontiguousarray(y.reshape(T, D).T, dtype=np.float32)
    ident = np.eye(128, dtype=np.float32)
    in_maps = []
    for c in range(N_CORES):
        sl = slice(c * DC, (c + 1) * DC)
        in_maps.append({
            "xT": xT,
            "yT": yT,
            "wqT": np.ascontiguousarray(Wq[sl].T * SCALE, dtype=np.float32),
            "wkT": np.ascontiguousarray(Wk[sl].T, dtype=np.float32),
            "wvT": np.ascontiguousarray(Wv[sl].T, dtype=np.float32),
            "woT": np.ascontiguousarray(Wo[:, sl].T, dtype=np.float32),
            "bq": np.ascontiguousarray((bq[sl] * SCALE).reshape(DC, 1), dtype=np.float32),
            "bk": np.ascontiguousarray(bk[sl].reshape(DC, 1), dtype=np.float32),
            "bv": np.ascontiguousarray(bv[sl].reshape(DC, 1), dtype=np.float32),
            "ident": ident,
            "vones": np.ones((128, 64), dtype=np.float32),
        })
    return in_maps


def _run(in_maps, trace=False):
    if os.environ.get("JAX_PLATFORMS", "").strip() == "cpu":
        os.environ.pop("JAX_PLATFORMS")
    nc = _get_nc()
    from concourse.bass_utils import run_bass_kernel_spmd
    return run_bass_kernel_spmd(nc, in_maps, core_ids=list(range(N_CORES)), trace=trace)


def _numpy_fallback(x, y, mask, Wq, bq, Wk, bk, Wv, bv, Wo, bo):
    Bs, Sq, Dm = x.shape
    H = 16
    q = (x @ Wq.T + bq).reshape(Bs, Sq, H, HD)
    k = (y @ Wk.T + bk).reshape(Bs, -1, H, HD)
    v = (y @ Wv.T + bv).reshape(Bs, -1, H, HD)
    score = np.einsum("bqhd,bkhd->bhqk", q, k) / np.sqrt(HD)
    score = score + (1.0 - mask[:, None, :, :]) * -1e9
    score -= score.max(axis=-1, keepdims=True)
    e = np.exp(score)
    attn = e / e.sum(axis=-1, keepdims=True)
    out = np.einsum("bhqk,bkhd->bqhd", attn, v).reshape(Bs, Sq, Dm)
    return (out @ Wo.T + bo).astype(np.float32)


def kernel(x, y, mask, Wq, bq, Wk, bk, Wv, bv, Wo, bo):
    x = np.asarray(x, dtype=np.float32)
    y = np.asarray(y, dtype=np.float32)
    mask = np.asarray(mask, dtype=np.float32)
    Wq = np.asarray(Wq, dtype=np.float32)
    bq = np.asarray(bq, dtype=np.float32)
    Wk = np.asarray(Wk, dtype=np.float32)
    bk = np.asarray(bk, dtype=np.float32)
    Wv = np.asarray(Wv, dtype=np.float32)
    bv = np.asarray(bv, dtype=np.float32)
    Wo = np.asarray(Wo, dtype=np.float32)
    bo = np.asarray(bo, dtype=np.float32)

    if not np.all(mask == 1.0):
        return _numpy_fallback(x, y, mask, Wq, bq, Wk, bk, Wv, bv, Wo, bo)

    in_maps = _prep_in_maps(x, y, Wq, bq, Wk, bk, Wv, bv, Wo)
    res = _run(in_maps, trace=False)
    total = res.results[0]["out"].astype(np.float32).copy()
    for c in range(1, N_CORES):
        total += res.results[c]["out"]
    total += bo
    return total.reshape(B, S, D).astype(np.float32)


# revision 33
# speedup vs baseline: 233.5111x; 1.2321x over previous
"""Multi-head attention (B=2, S=2048, D=1024, H=16) on 8 TRN2 NeuronCores.

Sharding: tensor-parallel over heads. Each core owns 2 heads (128 feature
channels): Wq/Wk/Wv sliced column-wise (rows of the [out,in] weight), Wo
sliced row-wise. x/y replicated. Each core emits a partial [4096, 1024]
output (its heads pushed through its Wo slice); host sums the 8 partials.

Device-side layout trick: everything is computed transposed-by-design so no
on-device transposes of the big activations are needed:
  - host feeds x^T, y^T  [1024, 4096]
  - Q^T/K^T = (W x^T)        [128 chan, 4096 tok]   (chan on partitions)
  - scores  S^T = K^T.T-slices @ Q^T  -> [kpos, q]  (softmax axis = partitions)
  - E = exp(S^T) unnormalized; denominator comes for free as a 65th "ones"
    column in the V stationary operand of the attn@V matmul
  - O_un^T [d, q] = [V|1].T @ E
  - normalize with a gpsimd partition-broadcast of 1/denom
  - out = O_norm^T.T-slices @ Wo^T   (q back on partitions)
Matmuls run as float32r (TF32-ish, 1 cycle/row at N>=256). Scores pack the
two heads into the PE array with row tiling (contraction=64 each).
"""

import os
import numpy as np
from contextlib import ExitStack

# Problem constants (hardcoded per contract; kernel.py must be self-contained)
B, S, D = 2, 2048, 1024
T = B * S            # 4096 flattened tokens
N_CORES = 8
DC = D // N_CORES    # 128 channels per core
HD = 64              # head dim
NH = DC // HD        # 2 heads per core
SCALE = 1.0 / np.sqrt(HD)  # folded into Wq/bq on host
KT_TILES = S // 128  # 16 key tiles per batch
QC = 512             # query chunk (matmul moving N)
NQC = S // QC        # 4 query chunks per batch

_CACHE = {}


def _get_nc(reps=1):
    key = f"nc{reps}"
    if key in _CACHE:
        return _CACHE[key]
    import concourse.bass as bass
    import concourse.mybir as mybir
    import concourse.tile as tile
    from concourse import bacc

    f32 = mybir.dt.float32
    f32r = mybir.dt.float32r
    PSUM = bass.MemorySpace.PSUM

    nc = bacc.Bacc(
        "TRN2",
        target_bir_lowering=False,
        debug=False,
        enable_asserts=False,
        num_devices=N_CORES,
    )

    xT_d = nc.dram_tensor("xT", [D, T], f32r, kind="ExternalInput").ap()
    yT_d = nc.dram_tensor("yT", [D, T], f32r, kind="ExternalInput").ap()
    wqT_d = nc.dram_tensor("wqT", [D, DC], f32r, kind="ExternalInput").ap()
    wkT_d = nc.dram_tensor("wkT", [D, DC], f32r, kind="ExternalInput").ap()
    wvT_d = nc.dram_tensor("wvT", [D, DC], f32r, kind="ExternalInput").ap()
    woT_d = nc.dram_tensor("woT", [DC, D], f32r, kind="ExternalInput").ap()
    bq_d = nc.dram_tensor("bq", [DC, 1], f32, kind="ExternalInput").ap()
    bk_d = nc.dram_tensor("bk", [DC, 1], f32, kind="ExternalInput").ap()
    bv_d = nc.dram_tensor("bv", [DC, 1], f32, kind="ExternalInput").ap()
    ident_d = nc.dram_tensor("ident", [128, 128], f32, kind="ExternalInput").ap()
    vones_d = nc.dram_tensor("vones", [128, 64], f32r, kind="ExternalInput").ap()
    out_d = nc.dram_tensor("out", [T, D], f32, kind="ExternalOutput").ap()

    VW = 2 * (HD + 1)  # 130: per key-tile V layout [V_h0 | 1 | V_h1 | 1]

    with tile.TileContext(nc) as tc, ExitStack() as top:
        persist = top.enter_context(tc.tile_pool(name="persist", bufs=1))

        # ---- persistent SBUF tensors ----
        wq_sb = persist.tile([128, D], f32r, tag="wq")    # [din-part, 8*128 chan]
        wk_sb = persist.tile([128, D], f32r, tag="wk")
        wv_sb = persist.tile([128, D], f32r, tag="wv")
        wo_sb = persist.tile([DC, D], f32r, tag="wo")     # [d-part, 1024 out]
        bq_sb = persist.tile([DC, 1], f32, tag="bq")
        bk_sb = persist.tile([DC, 1], f32, tag="bk")
        bv_sb = persist.tile([DC, 1], f32, tag="bv")
        id_sb = persist.tile([128, 128], f32, tag="ident")
        ones64_sb = persist.tile([1, HD], f32r, tag="ones64")
        qT_sb = persist.tile([DC, T], f32r, tag="qT")     # [chan, tok]
        kT_sb = persist.tile([DC, T], f32r, tag="kT")
        vT_sb = persist.tile([DC, T], f32, tag="vT")
        v_all = persist.tile([128, (T // 128) * VW], f32r, tag="vall")

        for i in range(D // 128):
            nc.sync.dma_start(wq_sb[:, i * 128:(i + 1) * 128], wqT_d[i * 128:(i + 1) * 128, :])
            nc.sync.dma_start(wk_sb[:, i * 128:(i + 1) * 128], wkT_d[i * 128:(i + 1) * 128, :])
            nc.sync.dma_start(wv_sb[:, i * 128:(i + 1) * 128], wvT_d[i * 128:(i + 1) * 128, :])
        nc.sync.dma_start(wo_sb[:], woT_d[:])
        nc.sync.dma_start(bq_sb[:], bq_d[:])
        nc.sync.dma_start(bk_sb[:], bk_d[:])
        nc.sync.dma_start(bv_sb[:], bv_d[:])
        nc.sync.dma_start(id_sb[:], ident_d[:])

        # ones columns of v_all (denominator rider rows)
        v3 = v_all[:].rearrange("p (t c) -> p t c", c=VW)
        vones_3d = vones_d[:, 0:T // 128].rearrange("p (t o) -> p t o", o=1)
        nc.sync.dma_start(v3[:, :, HD:HD + 1], vones_3d)
        nc.sync.dma_start(v3[:, :, 2 * HD + 1:2 * HD + 2], vones_3d)
        nc.sync.dma_start(ones64_sb[:], vones_d[0:1, 0:HD])

        # ---- phase 1: projections ----
        for _rep in range(reps):
            _build_body(nc, tc, mybir, bass, locals())

    nc.compile()
    _CACHE[key] = nc
    return nc


def _build_body(nc, tc, mybir, bass, env):
    f32 = mybir.dt.float32
    f32r = mybir.dt.float32r
    PSUM = bass.MemorySpace.PSUM
    xT_d, yT_d = env["xT_d"], env["yT_d"]
    out_d = env["out_d"]
    wq_sb, wk_sb, wv_sb, wo_sb = env["wq_sb"], env["wk_sb"], env["wv_sb"], env["wo_sb"]
    bq_sb, bk_sb, bv_sb = env["bq_sb"], env["bk_sb"], env["bv_sb"]
    id_sb, ones64_sb = env["id_sb"], env["ones64_sb"]
    qT_sb, kT_sb, vT_sb, v_all = env["qT_sb"], env["kT_sb"], env["vT_sb"], env["v_all"]
    VW = 2 * (HD + 1)

    if True:
        with ExitStack() as ph1:
            io_pool = ph1.enter_context(tc.tile_pool(name="io", bufs=6))
            pr_ps = ph1.enter_context(tc.tile_pool(name="prps", bufs=4, space=PSUM))
            vt_ps = ph1.enter_context(tc.tile_pool(name="vtps", bufs=2, space=PSUM))

            for tcn in range(T // QC):  # 8 token chunks of 512
                t0 = tcn * QC
                # Q^T chunk
                q_ps = pr_ps.tile([DC, QC], f32, tag="prps")
                for di in range(D // 128):
                    xt = io_pool.tile([128, QC], f32r, tag="io")
                    nc.sync.dma_start(xt[:], xT_d[di * 128:(di + 1) * 128, t0:t0 + QC])
                    nc.tensor.matmul(
                        q_ps[:],
                        wq_sb[:, di * 128:(di + 1) * 128],
                        xt[:],
                        start=(di == 0), stop=(di == D // 128 - 1),
                    )
                nc.vector.tensor_scalar_add(qT_sb[:, t0:t0 + QC], q_ps[:], bq_sb[:])
                # K^T and V^T chunks share the y tiles
                k_ps = pr_ps.tile([DC, QC], f32, tag="prps")
                v_ps = pr_ps.tile([DC, QC], f32, tag="prps")
                for di in range(D // 128):
                    yt = io_pool.tile([128, QC], f32r, tag="io")
                    nc.sync.dma_start(yt[:], yT_d[di * 128:(di + 1) * 128, t0:t0 + QC])
                    nc.tensor.matmul(
                        k_ps[:],
                        wk_sb[:, di * 128:(di + 1) * 128],
                        yt[:],
                        start=(di == 0), stop=(di == D // 128 - 1),
                    )
                    nc.tensor.matmul(
                        v_ps[:],
                        wv_sb[:, di * 128:(di + 1) * 128],
                        yt[:],
                        start=(di == 0), stop=(di == D // 128 - 1),
                    )
                nc.vector.tensor_scalar_add(kT_sb[:, t0:t0 + QC], k_ps[:], bk_sb[:])
                nc.vector.tensor_scalar_add(vT_sb[:, t0:t0 + QC], v_ps[:], bv_sb[:])

            # V^T -> V (PE transpose per 128x128 tile) into v_all
            for kt in range(T // 128):  # 32 key tiles over all tokens
                tp = vt_ps.tile([128, 128], f32, tag="vtps")
                nc.tensor.transpose(tp[:], vT_sb[:, kt * 128:(kt + 1) * 128], id_sb[:])
                for h in range(NH):
                    c0 = kt * VW + h * (HD + 1)
                    nc.vector.tensor_copy(v_all[:, c0:c0 + HD], tp[:, h * HD:(h + 1) * HD])

        # ---- phase 2: attention + Wo ----
        with ExitStack() as ph2:
            s_pool = ph2.enter_context(tc.tile_pool(name="sps", bufs=2, space=PSUM))
            o_pool = ph2.enter_context(tc.tile_pool(name="ops", bufs=2, space=PSUM))
            w_pool = ph2.enter_context(tc.tile_pool(name="wps", bufs=2, space=PSUM))
            e_pool = ph2.enter_context(tc.tile_pool(name="e", bufs=4))
            ou_pool = ph2.enter_context(tc.tile_pool(name="ou", bufs=3))
            rec_pool = ph2.enter_context(tc.tile_pool(name="rec", bufs=2))
            on_pool = ph2.enter_context(tc.tile_pool(name="on", bufs=2))
            st_pool = ph2.enter_context(tc.tile_pool(name="st", bufs=3))

            Exp = mybir.ActivationFunctionType.Exp
            Copy = mybir.ActivationFunctionType.Copy

            for b in range(B):
                for qc in range(NQC):
                    q0 = b * S + qc * QC
                    o_ps = [o_pool.tile([HD + 1, QC], f32, tag="ops", name=f"ops{b}_{qc}_{h}")
                            for h in range(NH)]
                    for ktp in range(KT_TILES // 2):
                        s_ps = []
                        for h in range(NH):
                            sp = s_pool.tile([128, 2 * QC], f32, tag="sps", name=f"sps{h}")
                            for j in range(2):
                                k0 = b * S + (2 * ktp + j) * 128
                                nc.tensor.matmul(
                                    sp[:, j * QC:(j + 1) * QC],
                                    kT_sb[h * HD:(h + 1) * HD, k0:k0 + 128],
                                    qT_sb[h * HD:(h + 1) * HD, q0:q0 + QC],
                                    start=True, stop=True,
                                    tile_position=(h * HD, 0),
                                )
                            s_ps.append(sp)
                        for h in range(NH):
                            e_sb = e_pool.tile([128, 2 * QC], f32r, tag="e")
                            nc.scalar.activation(e_sb[:], s_ps[h][:], Exp)
                            for j in range(2):
                                kt = 2 * ktp + j
                                c0 = (b * KT_TILES + kt) * VW + h * (HD + 1)
                                nc.tensor.matmul(
                                    o_ps[h][:],
                                    v_all[:, c0:c0 + HD + 1],
                                    e_sb[:, j * QC:(j + 1) * QC],
                                    start=(kt == 0), stop=(kt == KT_TILES - 1),
                                )
                    # normalize: O_norm^T[d, q] = O_un^T[d, q] / denom[q]
                    on_sb = on_pool.tile([DC, QC], f32r, tag="on")
                    for h in range(NH):
                        ou_sb = ou_pool.tile([HD + 1, QC], f32, tag="ou")
                        nc.vector.tensor_copy(ou_sb[:], o_ps[h][:])
                        rec_sb = rec_pool.tile([1, QC], f32r, tag="rec",
                                               name=f"rec{b}_{qc}_{h}")
                        with nc.allow_low_precision(reason="softmax denom recip to f32r"):
                            nc.vector.reciprocal(rec_sb[:], ou_sb[HD:HD + 1, :])
                        rb_ps = w_pool.tile([128, QC], f32, tag="wps",
                                            name=f"rb{b}_{qc}_{h}")
                        nc.tensor.matmul(rb_ps[0:HD, :], ones64_sb[:], rec_sb[:],
                                         start=True, stop=True)
                        nc.vector.tensor_mul(
                            on_sb[h * HD:(h + 1) * HD, :], ou_sb[0:HD, :],
                            rb_ps[0:HD, :]
                        )
                    # Wo: out[q, n] = sum_d O_norm^T[d, q] * woT[d, n]
                    for qs in range(QC // 128):
                        st = st_pool.tile([128, D], f32, tag="st")
                        for nn in range(D // QC):
                            wp = w_pool.tile([128, QC], f32, tag="wps", name=f"wp{nn}")
                            nc.tensor.matmul(
                                wp[:],
                                on_sb[:, qs * 128:(qs + 1) * 128],
                                wo_sb[:, nn * QC:(nn + 1) * QC],
                                start=True, stop=True,
                            )
                            nc.vector.tensor_copy(st[:, nn * QC:(nn + 1) * QC], wp[:])
                        r0 = q0 + qs * 128
                        nc.sync.dma_start(out_d[r0:r0 + 128, :], st[:])

def _prep_in_maps(x, y, Wq, bq, Wk, bk, Wv, bv, Wo):
    xT = np.ascontiguousarray(x.reshape(T, D).T, dtype=np.float32)
    yT = np.ascontiguousarray(y.reshape(T, D).T, dtype=np.float32)
    ident = np.eye(128, dtype=np.float32)
    in_maps = []
    for c in range(N_CORES):
        sl = slice(c * DC, (c + 1) * DC)
        in_maps.append({
            "xT": xT,
            "yT": yT,
            "wqT": np.ascontiguousarray(Wq[sl].T * SCALE, dtype=np.float32),
            "wkT": np.ascontiguousarray(Wk[sl].T, dtype=np.float32),
            "wvT": np.ascontiguousarray(Wv[sl].T, dtype=np.float32),
            "woT": np.ascontiguousarray(Wo[:, sl].T, dtype=np.float32),
            "bq": np.ascontiguousarray((bq[sl] * SCALE).reshape(DC, 1), dtype=np.float32),
            "bk": np.ascontiguousarray(bk[sl].reshape(DC, 1), dtype=np.float32),
            "bv": np.ascontiguousarray(bv[sl].reshape(DC, 1), dtype=np.float32),
            "ident": ident,
            "vones": np.ones((128, 64), dtype=np.float32),
        })
    return in_maps


def _run(in_maps, trace=False):
    if os.environ.get("JAX_PLATFORMS", "").strip() == "cpu":
        os.environ.pop("JAX_PLATFORMS")
    nc = _get_nc()
    from concourse.bass_utils import run_bass_kernel_spmd
    return run_bass_kernel_spmd(nc, in_maps, core_ids=list(range(N_CORES)), trace=trace)


def _numpy_fallback(x, y, mask, Wq, bq, Wk, bk, Wv, bv, Wo, bo):
    Bs, Sq, Dm = x.shape
    H = 16
    q = (x @ Wq.T + bq).reshape(Bs, Sq, H, HD)
    k = (y @ Wk.T + bk).reshape(Bs, -1, H, HD)
    v = (y @ Wv.T + bv).reshape(Bs, -1, H, HD)
    score = np.einsum("bqhd,bkhd->bhqk", q, k) / np.sqrt(HD)
    score = score + (1.0 - mask[:, None, :, :]) * -1e9
    score -= score.max(axis=-1, keepdims=True)
    e = np.exp(score)
    attn = e / e.sum(axis=-1, keepdims=True)
    out = np.einsum("bhqk,bkhd->bqhd", attn, v).reshape(Bs, Sq, Dm)
    return (out @ Wo.T + bo).astype(np.float32)


def kernel(x, y, mask, Wq, bq, Wk, bk, Wv, bv, Wo, bo):
    x = np.asarray(x, dtype=np.float32)
    y = np.asarray(y, dtype=np.float32)
    mask = np.asarray(mask, dtype=np.float32)
    Wq = np.asarray(Wq, dtype=np.float32)
    bq = np.asarray(bq, dtype=np.float32)
    Wk = np.asarray(Wk, dtype=np.float32)
    bk = np.asarray(bk, dtype=np.float32)
    Wv = np.asarray(Wv, dtype=np.float32)
    bv = np.asarray(bv, dtype=np.float32)
    Wo = np.asarray(Wo, dtype=np.float32)
    bo = np.asarray(bo, dtype=np.float32)

    if not np.all(mask == 1.0):
        return _numpy_fallback(x, y, mask, Wq, bq, Wk, bk, Wv, bv, Wo, bo)

    in_maps = _prep_in_maps(x, y, Wq, bq, Wk, bk, Wv, bv, Wo)
    res = _run(in_maps, trace=False)
    total = res.results[0]["out"].astype(np.float32).copy()
    for c in range(1, N_CORES):
        total += res.results[c]["out"]
    total += bo
    return total.reshape(B, S, D).astype(np.float32)
